# revision 15
# baseline (speedup 1.0000x reference)
"""Trainium2 Bass kernel for Mixtral-style GQA attention.

Full module: y = Attn(RoPE(hs@Wq), RoPE(hs@Wk), hs@Wv) @ Wo
  T=2048, HIDDEN=4096, 32 Q heads / 8 KV heads, head_dim=128, causal,
  neox rotate-half RoPE (base 1e6), fp32 in/out.

Sharding (8 cores, tensor-parallel over heads):
  core c: Q heads 4c..4c+3 (Wq cols c*512:+512), KV head c (Wk/Wv cols
  c*128:+128), Wo rows c*512:+512.  Each core computes a partial
  y^T [4096, 2048] in fp16; host sums the 8 partials and transposes.

bf16 design (all matmul operands bf16, PSUM fp32 accumulate):
  - Host pre-transposes hidden_states -> hst [4096, 2048] bf16, so H^T
    tiles stream straight from DRAM (no PE transposes, no ACT copies).
  - Host precomputes RoPE cos / +-sin tables [128, 2048] fp32.
  - Wq/Wk/Wv/Wo all SBUF-resident in bf16, loaded once in 0.5MB batches.
  - PE warmup transposes at t=0 lift the HAM clock gate to 2.4 GHz while
    the first weight/activation DMAs are in flight.
  - Phase P: Q^T/K^T/V^T = W^T @ H^T accumulated over 32 hid k-tiles
    (H^T streamed in 4-k-tile chunks); RoPE on the PSUM->SBUF drain.
    The q0 accumulator is double-buffered across s-groups and the six
    PSUM drains are spread over DVE/ACT in the order the next s-group's
    first matmuls need the banks back, so the boundary stall is ~0.
  - Phase A per (head, q-group of 512): S^T blocks [k,q] = K^T.T @ Q^T
    pipelined 3 deep, exp on ACT (scale fused, bf16 out), causal
    diagonal blocks trimmed to the live column range and masked by a
    DVE multiply with precomputed 0/1 tiles, row sums on DVE, PV with
    V-natural lhsT, normalize via reciprocal of the [1,512] sum row
    broadcast across partitions (gpsimd).
  - Phase O: y^T = Wo^T @ O^T accumulated over the 4 head tiles, fp16
    out; group g's out-proj matmuls are interleaved into group g+1's
    attention ahead of each j-block so the PE never starves on the exp
    chain; their PSUM drains all go to DVE to keep ACT exp-only.
"""
import math
import os

import numpy as np

import concourse.bass as bass
import concourse.mybir as mybir
import concourse.tile as tile
from concourse import bacc
from concourse.bass_utils import run_bass_kernel_spmd

F32 = mybir.dt.float32
F32R = mybir.dt.float32r
BF16 = mybir.dt.bfloat16
F16 = mybir.dt.float16
I32 = mybir.dt.int32
AF = mybir.ActivationFunctionType
ALU = mybir.AluOpType

T = 2048
HID = 4096
NH = 4            # q heads per core
D = 128           # head dim
DQ = NH * D       # 512
G = 512           # seq group size
NG = T // G       # 4
KT = HID // 128   # 32 hidden k-tiles
CH = 4            # k-tiles per H^T stream chunk
NCORES = 8
PD = 3            # attention S-block software pipeline depth

SCALE = 1.0 / math.sqrt(D)

LAST_EXEC_NS = None


def _emit(nc):
    hst = nc.dram_tensor("hst", [HID, T], BF16, kind="ExternalInput").ap()
    wq = nc.dram_tensor("wq", [HID, DQ], BF16, kind="ExternalInput").ap()
    wk = nc.dram_tensor("wk", [HID, D], BF16, kind="ExternalInput").ap()
    wv = nc.dram_tensor("wv", [HID, D], BF16, kind="ExternalInput").ap()
    wo = nc.dram_tensor("wo", [DQ, HID], BF16, kind="ExternalInput").ap()
    cosd = nc.dram_tensor("cosd", [128, T], F32, kind="ExternalInput").ap()
    sind = nc.dram_tensor("sind", [128, T], F32, kind="ExternalInput").ap()
    yt = nc.dram_tensor("yt", [HID, T], F16, kind="ExternalOutput").ap()

    hst_k = hst.rearrange("(a p) t -> p a t", p=128)   # [128, KT, T]
    wq_k = wq.rearrange("(a p) m -> p a m", p=128)     # [128, KT, DQ]
    wk_k = wk.rearrange("(a p) m -> p a m", p=128)     # [128, KT, D]
    wv_k = wv.rearrange("(a p) m -> p a m", p=128)

    with tile.TileContext(nc) as tc:
        with (
            tc.tile_pool(name="const", bufs=1) as const,
            tc.tile_pool(name="res", bufs=1) as res,
            tc.tile_pool(name="hp", bufs=3) as hp,
            tc.tile_pool(name="ro", bufs=2) as ro,
            tc.tile_pool(name="ex", bufs=6) as ex,
            tc.tile_pool(name="sc", bufs=2) as sc,
            tc.tile_pool(name="yo", bufs=4) as yo,
        ):
            # ------------- constants (staging pool closed after) -------------
            tmpc_cm = tc.tile_pool(name="tmpc", bufs=1)
            tmpc = tmpc_cm.__enter__()

            idf = tmpc.tile([128, 128], F32, name="idf", tag="idf")
            nc.gpsimd.memset(idf[:], 1.0)
            nc.gpsimd.affine_select(
                out=idf[:], in_=idf[:], compare_op=ALU.is_equal, fill=0.0,
                base=0, channel_multiplier=-1, pattern=[[1, 128]])
            identb = const.tile([128, 128], BF16, name="identb", tag="identb")
            nc.scalar.copy(identb[:], idf[:])

            # causal 0/1 masks for the 4 diagonal-block offsets:
            # mask4[p, o, q] = 1 if q >= p + o*128 else 0
            maskf = tmpc.tile([128, 4, G], F32, name="maskf", tag="maskf")
            nc.gpsimd.memset(maskf[:], 1.0)
            for o in range(4):
                nc.gpsimd.affine_select(
                    out=maskf[:, o, :], in_=maskf[:, o, :],
                    compare_op=ALU.is_ge, fill=0.0,
                    base=-o * 128, channel_multiplier=-1, pattern=[[1, G]])
            mask4 = const.tile([128, 4, G], BF16, name="mask4", tag="mask4")
            nc.scalar.copy(mask4[:], maskf[:])
            zf = tmpc.tile([128, G], F32, name="zf", tag="zf")
            nc.gpsimd.memset(zf[:], 0.0)
            zb = const.tile([128, G], BF16, name="zb", tag="zb")
            nc.scalar.copy(zb[:], zf[:])

            tmpc_cm.__exit__(None, None, None)

            # resident weights and rope tables
            wq_sb = res.tile([128, KT, DQ], BF16, name="wq_sb", tag="wq_sb")
            wk_sb = res.tile([128, KT, D], BF16, name="wk_sb", tag="wk_sb")
            wv_sb = res.tile([128, KT, D], BF16, name="wv_sb", tag="wv_sb")
            wo_sb = res.tile([128, NH, HID], BF16, name="wo_sb", tag="wo_sb")
            cosf = res.tile([128, T], F32, name="cosf", tag="cosf")
            sinpm = res.tile([128, T], F32, name="sinpm", tag="sinpm")

            # resident activations (qt also doubles as O^T after attention)
            qt = [res.tile([128, T], BF16, name=f"qt{h}", tag=f"qt{h}")
                  for h in range(NH)]
            kt = res.tile([128, T], BF16, name="kt", tag="kt")
            vnat = res.tile([128, T // 128, D], BF16, name="vnat", tag="vnat")

            # ---------------- phase P: projections ----------------
            with (
                tc.tile_pool(name="accp", bufs=1, space="PSUM") as accp,
                tc.tile_pool(name="tpp", bufs=1, space="PSUM") as tpp,
            ):
                # spin the PE while the first DMAs land so HAM unthrottles
                warm_tp = tpp.tile([128, G], BF16, name="tp", tag="tp")
                for _ in range(32):
                    nc.tensor.transpose(warm_tp[:, 0:128], identb[:],
                                        identb[:])

                for s in range(NG):
                    ssl = bass.ts(s, G)
                    q_ps = [accp.tile([128, G], F32, name=f"qps{f}",
                                      tag=f"qps{f}",
                                      bufs=2 if f == 0 else 1)
                            for f in range(NH)]
                    k_ps = accp.tile([128, G], F32, name="kps", tag="kps")
                    v_ps = accp.tile([128, G], F32, name="vps", tag="vps")

                    for c in range(KT // CH):
                        csl2 = bass.ds(c * CH, CH)
                        if s == 0:
                            nc.sync.dma_start(wq_sb[:, csl2, :],
                                              wq_k[:, csl2, :])
                        htc = hp.tile([128, CH, G], BF16, name="htc",
                                      tag="htc")
                        nc.sync.dma_start(htc[:], hst_k[:, csl2, ssl])
                        if s == 0:
                            nc.sync.dma_start(wk_sb[:, csl2, :],
                                              wk_k[:, csl2, :])
                            nc.sync.dma_start(wv_sb[:, csl2, :],
                                              wv_k[:, csl2, :])
                            if c == 1:
                                nc.sync.dma_start(cosf[:], cosd[:, :])
                                nc.sync.dma_start(sinpm[:], sind[:, :])
                        for kk in range(CH):
                            k = c * CH + kk
                            ht = htc[:, kk, :]
                            st = (k == 0)
                            sp = (k == KT - 1)
                            # bank order must match the drain schedule below
                            nc.tensor.matmul(q_ps[0][:], wq_sb[:, k, 0:128],
                                             ht, start=st, stop=sp)
                            nc.tensor.matmul(k_ps[:], wk_sb[:, k, :], ht,
                                             start=st, stop=sp)
                            nc.tensor.matmul(v_ps[:], wv_sb[:, k, :], ht,
                                             start=st, stop=sp)
                            nc.tensor.matmul(q_ps[2][:],
                                             wq_sb[:, k, 256:384],
                                             ht, start=st, stop=sp)
                            nc.tensor.matmul(q_ps[3][:],
                                             wq_sb[:, k, 384:512],
                                             ht, start=st, stop=sp)
                            nc.tensor.matmul(q_ps[1][:],
                                             wq_sb[:, k, 128:256],
                                             ht, start=st, stop=sp)

                    # drain all six accumulators first, on the engines and
                    # in the order the next s-group's matmuls reuse banks:
                    # q0 is double-buffered (no rush), then k,v,q2,q3 on
                    # DVE (267ns each), q1 then q0 on ACT.
                    raws = {}
                    for x, eng in ((NH, 'v'), (1, 'v'), (2, 'v'), (3, 'v'),
                                   (NH + 1, 'v'), (0, 's')):
                        src = (q_ps[x] if x < NH else
                               (k_ps if x == NH else v_ps))
                        if x == NH + 1:
                            vraw = ro.tile([128, G], BF16, name="vraw",
                                           tag="vraw", bufs=1)
                            nc.vector.tensor_copy(vraw[:], src[:])
                            continue
                        raw = ro.tile([128, G], F32, name="raw", tag="raw",
                                      bufs=6)
                        if eng == 'v':
                            nc.vector.tensor_copy(raw[:], src[:])
                        else:
                            nc.scalar.copy(raw[:], src[:])
                        raws[x] = raw

                    # v: PE-transpose to natural layout
                    tpv = tpp.tile([128, G], BF16, name="tp", tag="tp")
                    for sub in range(4):
                        nc.tensor.transpose(
                            tpv[:, sub * 128:(sub + 1) * 128],
                            vraw[:, sub * 128:(sub + 1) * 128], identb[:])
                    nc.scalar.copy(vnat[:, 4 * s:4 * s + 4, :], tpv[:])

                    # RoPE for q heads + k
                    for x in range(NH + 1):
                        raw = raws[x]
                        dst = qt[x][:, ssl] if x < NH else kt[:, ssl]
                        rot = ro.tile([128, G], F32, name="rot", tag="rot")
                        nc.gpsimd.dma_start(rot[0:64, :], raw[64:128, :])
                        nc.gpsimd.dma_start(rot[64:128, :], raw[0:64, :])
                        tmp = ro.tile([128, G], F32, name="tmp", tag="tmp",
                                      bufs=1)
                        nc.vector.tensor_mul(tmp[:], rot[:], sinpm[:, ssl])
                        nc.vector.tensor_mul(dst, raw[:], cosf[:, ssl])
                        nc.vector.tensor_add(dst, dst, tmp[:])

            # out-proj weights: emitted after P so the big ht/weight loads
            # win the DMA queues early; needed only at first Y phase.
            nc.sync.dma_start(wo_sb[:], wo.rearrange("(f p) j -> p f j", p=128))

            # ---------------- phase A + O interleaved ----------------
            with (
                tc.tile_pool(name="pss", bufs=4, space="PSUM") as pss,
                tc.tile_pool(name="pso", bufs=2, space="PSUM") as pso,
                tc.tile_pool(name="psy", bufs=2, space="PSUM") as psy,
            ):
                def emit_y(gy, m):
                    gsl = bass.ts(gy, G)
                    y_ps = psy.tile([128, G], F32, name="yps", tag="yps")
                    for f in range(NH):
                        nc.tensor.matmul(
                            y_ps[:], wo_sb[:, f, m * 128:(m + 1) * 128],
                            qt[f][:, gsl],
                            start=(f == 0), stop=(f == NH - 1))
                    y_sb = yo.tile([128, G], F16, name="ysb", tag="ysb")
                    if m % 2 == 0:
                        nc.scalar.copy(y_sb[:], y_ps[:])
                    else:
                        nc.vector.tensor_copy(y_sb[:], y_ps[:])
                    nc.sync.dma_start(yt[m * 128:(m + 1) * 128, gsl], y_sb[:])

                for g in range(NG):
                    jn = 4 * g + 4
                    tj = NH * jn       # attention j-blocks in this group
                    jdone = 0
                    ym = 0             # Y_{g-1} m-tiles emitted so far
                    for h in range(NH):
                        gsl = bass.ts(g, G)
                        o_ps = pso.tile([128, G], F32, name="ops", tag="ops")
                        # softmax denominators accumulate on gpsimd, the
                        # one engine with slack; it also owns the final
                        # partition-axis reduce.  Leaf chain — nothing on
                        # the PE waits for it until the normalize.
                        sumacc = sc.tile([128, G], F32, name="sumacc",
                                         tag="sumacc")

                        s_tiles = {}

                        def emit_s(j, h=h, g=g):
                            s_ps = pss.tile([128, G], F32, name="sps",
                                            tag="sps")
                            qs = max(0, (j - 4 * g) * 128)
                            nc.tensor.matmul(
                                s_ps[:, qs:], kt[:, j * 128:(j + 1) * 128],
                                qt[h][:, g * G + qs:(g + 1) * G],
                                start=True, stop=True)
                            s_tiles[j] = s_ps

                        for j in range(min(PD, jn)):
                            emit_s(j)
                        for j in range(jn):
                            # out-proj of the previous group rides ahead of
                            # this j-block to keep the PE fed while ACT works
                            if g >= 1:
                                while ym < 32 and ym * tj < 32 * (jdone + 1):
                                    emit_y(g - 1, ym)
                                    ym += 1
                            s_ps = s_tiles.pop(j)
                            o = j - 4 * g
                            qs = max(0, o * 128)
                            e_sb = ex.tile([128, G], BF16, name="esb",
                                           tag="esb")
                            if qs:
                                nc.vector.tensor_copy(e_sb[:, :qs],
                                                      zb[:, :qs])
                            nc.scalar.activation(e_sb[:, qs:], s_ps[:, qs:],
                                                 AF.Exp, scale=SCALE)
                            if o >= 0:
                                # causal mask via 0/1 multiply (DVE) — keeps
                                # gpsimd off the exp->PV chain
                                nc.vector.tensor_mul(e_sb[:, qs:],
                                                     e_sb[:, qs:],
                                                     mask4[:, o, qs:])
                            if j + PD < jn:
                                emit_s(j + PD)
                            if j == 0:
                                nc.gpsimd.tensor_copy(sumacc[:], e_sb[:])
                            else:
                                nc.gpsimd.tensor_add(sumacc[:], sumacc[:],
                                                     e_sb[:])
                            nc.tensor.matmul(o_ps[:], vnat[:, j, :], e_sb[:],
                                             start=(j == 0), stop=(j == jn - 1))
                            jdone += 1
                        # partition-axis reduce -> [1, G] denominators, then
                        # reciprocal, broadcast back across partitions.
                        s_row = sc.tile([1, G], F32, name="srow", tag="srow")
                        nc.gpsimd.tensor_reduce(s_row[:], sumacc[:],
                                                axis=mybir.AxisListType.C,
                                                op=ALU.add)
                        s_rec = sc.tile([1, G], F32, name="srec", tag="srec")
                        nc.vector.reciprocal(s_rec[:], s_row[:])
                        recb = sc.tile([128, G], F32, name="recb", tag="recb")
                        nc.gpsimd.partition_broadcast(recb[:], s_rec[:])
                        # normalized O^T drains straight into qt[h] (bf16)
                        nc.vector.tensor_mul(qt[h][:, gsl], o_ps[:], recb[:])
                    if g >= 1:
                        while ym < 32:
                            emit_y(g - 1, ym)
                            ym += 1
                # final group's out-projection
                for m in range(KT):
                    emit_y(NG - 1, m)
    return nc


_NC_CACHE = None


def _get_nc():
    global _NC_CACHE
    if _NC_CACHE is None:
        nc = bacc.Bacc("TRN2", target_bir_lowering=False, debug=False,
                       num_devices=NCORES)
        _emit(nc)
        nc.compile()
        _NC_CACHE = nc
    return _NC_CACHE


def _install_ntff_hook():
    import sys
    import types
    try:
        import trn_agent_boot.trn_boot as tb
        hook = tb._ntff_profile_via_ctypes('/opt/axon/libaxon_pjrt.so')
        if hook is None:
            return
        mod = types.ModuleType('antenv.axon_hooks')
        mod.get_axon_ntff_profile_hook = lambda: hook
        sys.modules['antenv.axon_hooks'] = mod
    except Exception:
        pass


def kernel(**inputs):
    global LAST_EXEC_NS
    import ml_dtypes
    BF = ml_dtypes.bfloat16

    positions = np.asarray(inputs["positions"]).astype(np.float64)
    hidden = np.asarray(inputs["hidden_states"], dtype=np.float32)
    Wq = np.asarray(inputs["Wq"], dtype=np.float32)
    Wk = np.asarray(inputs["Wk"], dtype=np.float32)
    Wv = np.asarray(inputs["Wv"], dtype=np.float32)
    Wo = np.asarray(inputs["Wo"], dtype=np.float32)

    hst = np.ascontiguousarray(hidden.T).astype(BF)

    # neox rotate-half RoPE tables, partition p carries frequency p & 63;
    # top half gets -sin so that raw*cos + rot*sinpm == rotate_half rope.
    p = np.arange(128)
    invf = (1e6) ** (-(p & 63) / 64.0)
    ang = invf[:, None] * positions[None, :]
    cosd = np.cos(ang).astype(np.float32)
    sin = np.sin(ang)
    sind = np.concatenate([-sin[:64], sin[64:]], axis=0).astype(np.float32)

    trace = os.environ.get("KERNEL_TRACE", "0") == "1"
    if trace:
        _install_ntff_hook()

    nc = _get_nc()
    in_maps = []
    for c in range(NCORES):
        in_maps.append({
            "hst": hst,
            "wq": np.ascontiguousarray(Wq[:, c * DQ:(c + 1) * DQ]).astype(BF),
            "wk": np.ascontiguousarray(Wk[:, c * D:(c + 1) * D]).astype(BF),
            "wv": np.ascontiguousarray(Wv[:, c * D:(c + 1) * D]).astype(BF),
            "wo": np.ascontiguousarray(Wo[c * DQ:(c + 1) * DQ, :]).astype(BF),
            "cosd": cosd,
            "sind": sind,
        })
    res = run_bass_kernel_spmd(nc, in_maps, core_ids=list(range(NCORES)),
                               trace=trace)
    LAST_EXEC_NS = res.exec_time_ns
    acc = np.zeros((HID, T), dtype=np.float32)
    for c in range(NCORES):
        acc += res.results[c]["yt"].astype(np.float32)
    return np.ascontiguousarray(acc.T)


# revision 19
# speedup vs baseline: 4.0783x; 4.0783x over previous
"""Trainium2 Bass kernel for Mixtral-style GQA attention.

Full module: y = Attn(RoPE(hs@Wq), RoPE(hs@Wk), hs@Wv) @ Wo
  T=2048, HIDDEN=4096, 32 Q heads / 8 KV heads, head_dim=128, causal,
  neox rotate-half RoPE (base 1e6), fp32 in/out.

Sharding (8 cores, tensor-parallel over heads):
  core c: Q heads 4c..4c+3 (Wq cols c*512:+512), KV head c (Wk/Wv cols
  c*128:+128), Wo rows c*512:+512.  Each core computes a partial
  y^T [4096, 2048] in fp16; host sums the 8 partials and transposes.

bf16 design (all matmul operands bf16, PSUM fp32 accumulate):
  - Host pre-transposes hidden_states -> hst [4096, 2048] bf16, so H^T
    tiles stream straight from DRAM (no PE transposes, no ACT copies).
  - Host precomputes RoPE cos / +-sin tables [128, 2048] fp32.
  - Wq/Wk/Wv/Wo all SBUF-resident in bf16, loaded once in 0.5MB batches.
  - PE warmup transposes at t=0 lift the HAM clock gate to 2.4 GHz while
    the first weight/activation DMAs are in flight.
  - Phase P: Q^T/K^T/V^T = W^T @ H^T accumulated over 32 hid k-tiles
    (H^T streamed in 4-k-tile chunks); RoPE on the PSUM->SBUF drain.
    The q0 accumulator is double-buffered across s-groups and the six
    PSUM drains are spread over DVE/ACT in the order the next s-group's
    first matmuls need the banks back, so the boundary stall is ~0.
  - Phase A per (head, q-group of 512): S^T blocks [k,q] = K^T.T @ Q^T
    pipelined 3 deep, exp on ACT (scale fused, bf16 out), causal
    diagonal blocks trimmed to the live column range and masked by a
    DVE multiply with precomputed 0/1 tiles, row sums on DVE, PV with
    V-natural lhsT, normalize via reciprocal of the [1,512] sum row
    broadcast across partitions (gpsimd).
  - Phase O: y^T = Wo^T @ O^T accumulated over the 4 head tiles, fp16
    out; group g's out-proj matmuls are interleaved into group g+1's
    attention ahead of each j-block so the PE never starves on the exp
    chain; their PSUM drains all go to DVE to keep ACT exp-only.
"""
import math
import os

import numpy as np

import concourse.bass as bass
import concourse.mybir as mybir
import concourse.tile as tile
from concourse import bacc
from concourse.bass_utils import run_bass_kernel_spmd

F32 = mybir.dt.float32
F32R = mybir.dt.float32r
BF16 = mybir.dt.bfloat16
F16 = mybir.dt.float16
I32 = mybir.dt.int32
AF = mybir.ActivationFunctionType
ALU = mybir.AluOpType

T = 2048
HID = 4096
NH = 4            # q heads per core
D = 128           # head dim
DQ = NH * D       # 512
G = 512           # seq group size
NG = T // G       # 4
KT = HID // 128   # 32 hidden k-tiles
CH = 4            # k-tiles per H^T stream chunk
NCORES = 8
PD = 3            # attention S-block software pipeline depth

SCALE = 1.0 / math.sqrt(D)

LAST_EXEC_NS = None


def _emit(nc):
    hst = nc.dram_tensor("hst", [HID, T], BF16, kind="ExternalInput").ap()
    wq = nc.dram_tensor("wq", [HID, DQ], BF16, kind="ExternalInput").ap()
    wk = nc.dram_tensor("wk", [HID, D], BF16, kind="ExternalInput").ap()
    wv = nc.dram_tensor("wv", [HID, D], BF16, kind="ExternalInput").ap()
    wo = nc.dram_tensor("wo", [DQ, HID], BF16, kind="ExternalInput").ap()
    cosd = nc.dram_tensor("cosd", [128, T], F32, kind="ExternalInput").ap()
    sind = nc.dram_tensor("sind", [128, T], F32, kind="ExternalInput").ap()
    yt = nc.dram_tensor("yt", [HID, T], F16, kind="ExternalOutput").ap()

    hst_k = hst.rearrange("(a p) t -> p a t", p=128)   # [128, KT, T]
    wq_k = wq.rearrange("(a p) m -> p a m", p=128)     # [128, KT, DQ]
    wk_k = wk.rearrange("(a p) m -> p a m", p=128)     # [128, KT, D]
    wv_k = wv.rearrange("(a p) m -> p a m", p=128)

    with tile.TileContext(nc) as tc:
        with (
            tc.tile_pool(name="const", bufs=1) as const,
            tc.tile_pool(name="res", bufs=1) as res,
            tc.tile_pool(name="hp", bufs=3) as hp,
            tc.tile_pool(name="ro", bufs=2) as ro,
            tc.tile_pool(name="ex", bufs=6) as ex,
            tc.tile_pool(name="sc", bufs=2) as sc,
            tc.tile_pool(name="yo", bufs=4) as yo,
        ):
            # ------------- constants (staging pool closed after) -------------
            tmpc_cm = tc.tile_pool(name="tmpc", bufs=1)
            tmpc = tmpc_cm.__enter__()

            idf = tmpc.tile([128, 128], F32, name="idf", tag="idf")
            nc.gpsimd.memset(idf[:], 1.0)
            nc.gpsimd.affine_select(
                out=idf[:], in_=idf[:], compare_op=ALU.is_equal, fill=0.0,
                base=0, channel_multiplier=-1, pattern=[[1, 128]])
            identh = const.tile([128, 128], F16, name="identh", tag="identh")
            nc.scalar.copy(identh[:], idf[:])

            zf = tmpc.tile([128, G], F32, name="zf", tag="zf")
            nc.gpsimd.memset(zf[:], 0.0)
            zb = const.tile([128, G], F16, name="zb", tag="zb")
            nc.scalar.copy(zb[:], zf[:])

            onesf = tmpc.tile([128, 1], F32, name="onesf", tag="onesf")
            nc.gpsimd.memset(onesf[:], 1.0)
            ones = const.tile([128, 1], F16, name="ones", tag="ones")
            nc.scalar.copy(ones[:], onesf[:])

            tmpc_cm.__exit__(None, None, None)

            # resident weights and rope tables
            wq_sb = res.tile([128, KT, DQ], BF16, name="wq_sb", tag="wq_sb")
            wk_sb = res.tile([128, KT, D], BF16, name="wk_sb", tag="wk_sb")
            wv_sb = res.tile([128, KT, D], BF16, name="wv_sb", tag="wv_sb")
            wo_sb = res.tile([128, NH, HID], BF16, name="wo_sb", tag="wo_sb")
            cosf = res.tile([128, T], F32, name="cosf", tag="cosf")
            sinpm = res.tile([128, T], F32, name="sinpm", tag="sinpm")

            # resident activations (qt also doubles as O^T after attention)
            qt = [res.tile([128, T], BF16, name=f"qt{h}", tag=f"qt{h}")
                  for h in range(NH)]
            kt = res.tile([128, T], BF16, name="kt", tag="kt")
            vnat = res.tile([128, T // 128, D], F16, name="vnat", tag="vnat")

            # ---------------- phase P: projections ----------------
            with (
                tc.tile_pool(name="accp", bufs=1, space="PSUM") as accp,
                tc.tile_pool(name="tpp", bufs=1, space="PSUM") as tpp,
            ):
                # spin the PE while the first DMAs land so HAM unthrottles
                warm_tp = tpp.tile([128, G], F16, name="tph", tag="tph")
                for _ in range(32):
                    nc.tensor.transpose(warm_tp[:, 0:128], identh[:],
                                        identh[:])

                for s in range(NG):
                    ssl = bass.ts(s, G)
                    q_ps = [accp.tile([128, G], F32, name=f"qps{f}",
                                      tag=f"qps{f}",
                                      bufs=2 if f == 0 else 1)
                            for f in range(NH)]
                    k_ps = accp.tile([128, G], F32, name="kps", tag="kps")
                    v_ps = accp.tile([128, G], F32, name="vps", tag="vps")

                    for c in range(KT // CH):
                        csl2 = bass.ds(c * CH, CH)
                        if s == 0:
                            nc.sync.dma_start(wq_sb[:, csl2, :],
                                              wq_k[:, csl2, :])
                        htc = hp.tile([128, CH, G], BF16, name="htc",
                                      tag="htc")
                        nc.sync.dma_start(htc[:], hst_k[:, csl2, ssl])
                        if s == 0:
                            nc.sync.dma_start(wk_sb[:, csl2, :],
                                              wk_k[:, csl2, :])
                            nc.sync.dma_start(wv_sb[:, csl2, :],
                                              wv_k[:, csl2, :])
                            if c == 1:
                                nc.sync.dma_start(cosf[:], cosd[:, :])
                                nc.sync.dma_start(sinpm[:], sind[:, :])
                        for kk in range(CH):
                            k = c * CH + kk
                            ht = htc[:, kk, :]
                            st = (k == 0)
                            sp = (k == KT - 1)
                            # bank order must match the drain schedule below
                            nc.tensor.matmul(q_ps[0][:], wq_sb[:, k, 0:128],
                                             ht, start=st, stop=sp)
                            nc.tensor.matmul(k_ps[:], wk_sb[:, k, :], ht,
                                             start=st, stop=sp)
                            nc.tensor.matmul(v_ps[:], wv_sb[:, k, :], ht,
                                             start=st, stop=sp)
                            nc.tensor.matmul(q_ps[2][:],
                                             wq_sb[:, k, 256:384],
                                             ht, start=st, stop=sp)
                            nc.tensor.matmul(q_ps[3][:],
                                             wq_sb[:, k, 384:512],
                                             ht, start=st, stop=sp)
                            nc.tensor.matmul(q_ps[1][:],
                                             wq_sb[:, k, 128:256],
                                             ht, start=st, stop=sp)

                    # drain all six accumulators first, on the engines and
                    # in the order the next s-group's matmuls reuse banks:
                    # q0 is double-buffered (no rush), then k,v,q2,q3 on
                    # DVE (267ns each), q1 then q0 on ACT.
                    raws = {}
                    for x, eng in ((NH, 'v'), (1, 'v'), (2, 'v'), (3, 'v'),
                                   (NH + 1, 'v'), (0, 's')):
                        src = (q_ps[x] if x < NH else
                               (k_ps if x == NH else v_ps))
                        if x == NH + 1:
                            vraw = ro.tile([128, G], F16, name="vraw",
                                           tag="vraw", bufs=1)
                            nc.vector.tensor_copy(vraw[:], src[:])
                            continue
                        raw = ro.tile([128, G], F32, name="raw", tag="raw",
                                      bufs=6)
                        if eng == 'v':
                            nc.vector.tensor_copy(raw[:], src[:])
                        else:
                            nc.scalar.copy(raw[:], src[:])
                        raws[x] = raw

                    # v: PE-transpose to natural layout
                    tpv = tpp.tile([128, G], F16, name="tph", tag="tph")
                    for sub in range(4):
                        nc.tensor.transpose(
                            tpv[:, sub * 128:(sub + 1) * 128],
                            vraw[:, sub * 128:(sub + 1) * 128], identh[:])
                    nc.scalar.copy(vnat[:, 4 * s:4 * s + 4, :], tpv[:])

                    # RoPE for q heads + k
                    for x in range(NH + 1):
                        raw = raws[x]
                        dst = qt[x][:, ssl] if x < NH else kt[:, ssl]
                        rot = ro.tile([128, G], F32, name="rot", tag="rot")
                        nc.gpsimd.dma_start(rot[0:64, :], raw[64:128, :])
                        nc.gpsimd.dma_start(rot[64:128, :], raw[0:64, :])
                        tmp = ro.tile([128, G], F32, name="tmp", tag="tmp",
                                      bufs=1)
                        nc.vector.tensor_mul(tmp[:], rot[:], sinpm[:, ssl])
                        nc.vector.tensor_mul(dst, raw[:], cosf[:, ssl])
                        nc.vector.tensor_add(dst, dst, tmp[:])

            # out-proj weights: emitted after P so the big ht/weight loads
            # win the DMA queues early; needed only at first Y phase.
            nc.sync.dma_start(wo_sb[:], wo.rearrange("(f p) j -> p f j", p=128))

            # ---------------- phase A + O interleaved ----------------
            with (
                tc.tile_pool(name="pss", bufs=3, space="PSUM") as pss,
                tc.tile_pool(name="pssum", bufs=1, space="PSUM") as pssum,
                tc.tile_pool(name="pso", bufs=2, space="PSUM") as pso,
                tc.tile_pool(name="psy", bufs=2, space="PSUM") as psy,
            ):
                def emit_y(gy, m):
                    gsl = bass.ts(gy, G)
                    y_ps = psy.tile([128, G], F32, name="yps", tag="yps")
                    for f in range(NH):
                        nc.tensor.matmul(
                            y_ps[:], wo_sb[:, f, m * 128:(m + 1) * 128],
                            qt[f][:, gsl],
                            start=(f == 0), stop=(f == NH - 1))
                    y_sb = yo.tile([128, G], F16, name="ysb", tag="ysb")
                    if m % 2 == 0:
                        nc.scalar.copy(y_sb[:], y_ps[:])
                    else:
                        nc.vector.tensor_copy(y_sb[:], y_ps[:])
                    nc.sync.dma_start(yt[m * 128:(m + 1) * 128, gsl], y_sb[:])

                for g in range(NG):
                    jn = 4 * g + 4
                    tj = NH * jn       # attention j-blocks in this group
                    jdone = 0
                    ym = 0             # Y_{g-1} m-tiles emitted so far
                    for h in range(NH):
                        gsl = bass.ts(g, G)
                        o_ps = pso.tile([128, G], F32, name="ops", tag="ops")
                        # softmax denominators accumulate in fp16 on the
                        # DVE (2x 16-bit rate, 10 mantissa bits keeps the
                        # accumulation error ~0.1%); one final ones-matmul
                        # reduces over partitions.
                        sumacc = sc.tile([128, G], F16, name="sumacc",
                                         tag="sumacc")

                        s_tiles = {}

                        def emit_s(j, h=h, g=g):
                            s_ps = pss.tile([128, G], F32, name="sps",
                                            tag="sps")
                            qs = max(0, (j - 4 * g) * 128)
                            nc.tensor.matmul(
                                s_ps[:, qs:], kt[:, j * 128:(j + 1) * 128],
                                qt[h][:, g * G + qs:(g + 1) * G],
                                start=True, stop=True)
                            s_tiles[j] = s_ps

                        for j in range(min(PD, jn)):
                            emit_s(j)
                        for j in range(jn):
                            # out-proj of the previous group rides ahead of
                            # this j-block to keep the PE fed while ACT works
                            if g >= 1:
                                while ym < 32 and ym * tj < 32 * (jdone + 1):
                                    emit_y(g - 1, ym)
                                    ym += 1
                            s_ps = s_tiles.pop(j)
                            o = j - 4 * g
                            qs = max(0, o * 128)
                            e_sb = ex.tile([128, G], F16, name="esb",
                                           tag="esb")
                            if qs:
                                nc.scalar.copy(e_sb[:, :qs], zb[:, :qs])
                            nc.scalar.activation(e_sb[:, qs:], s_ps[:, qs:],
                                                 AF.Exp, scale=SCALE)
                            if o >= 0:
                                # causal mask on gpsimd (idle engine)
                                nc.gpsimd.affine_select(
                                    out=e_sb[:, qs:], in_=e_sb[:, qs:],
                                    compare_op=ALU.is_ge, fill=0.0,
                                    base=0, channel_multiplier=-1,
                                    pattern=[[1, G - qs]])
                            if j + PD < jn:
                                emit_s(j + PD)
                            if j == 0:
                                nc.vector.tensor_copy(sumacc[:], e_sb[:])
                            else:
                                nc.vector.tensor_add(sumacc[:], sumacc[:],
                                                     e_sb[:])
                            nc.tensor.matmul(o_ps[:], vnat[:, j, :], e_sb[:],
                                             start=(j == 0), stop=(j == jn - 1))
                            jdone += 1
                        # ones^T @ sumacc -> [1, G] denominators on the PE,
                        # then fast-approx reciprocal (~18 bits, plenty).
                        s_sum = pssum.tile([1, G], F32, name="ssum",
                                           tag="ssum")
                        nc.tensor.matmul(s_sum[:], ones[:], sumacc[:],
                                         start=True, stop=True)
                        s_rec = sc.tile([1, G], F32, name="srec", tag="srec")
                        nc.vector.reciprocal_approx_fast(s_rec[:], s_sum[:])
                        recb = sc.tile([128, G], F32, name="recb", tag="recb")
                        nc.gpsimd.partition_broadcast(recb[:], s_rec[:])
                        # normalized O^T drains straight into qt[h] (bf16)
                        nc.vector.tensor_mul(qt[h][:, gsl], o_ps[:], recb[:])
                    if g >= 1:
                        while ym < 32:
                            emit_y(g - 1, ym)
                            ym += 1
                # final group's out-projection
                for m in range(KT):
                    emit_y(NG - 1, m)
    return nc


_NC_CACHE = None


def _get_nc():
    global _NC_CACHE
    if _NC_CACHE is None:
        nc = bacc.Bacc("TRN2", target_bir_lowering=False, debug=False,
                       num_devices=NCORES)
        _emit(nc)
        nc.compile()
        _NC_CACHE = nc
    return _NC_CACHE


def _install_ntff_hook():
    import sys
    import types
    try:
        import trn_agent_boot.trn_boot as tb
        hook = tb._ntff_profile_via_ctypes('/opt/axon/libaxon_pjrt.so')
        if hook is None:
            return
        mod = types.ModuleType('antenv.axon_hooks')
        mod.get_axon_ntff_profile_hook = lambda: hook
        sys.modules['antenv.axon_hooks'] = mod
    except Exception:
        pass


def kernel(**inputs):
    global LAST_EXEC_NS
    import ml_dtypes
    BF = ml_dtypes.bfloat16

    positions = np.asarray(inputs["positions"]).astype(np.float64)
    hidden = np.asarray(inputs["hidden_states"], dtype=np.float32)
    Wq = np.asarray(inputs["Wq"], dtype=np.float32)
    Wk = np.asarray(inputs["Wk"], dtype=np.float32)
    Wv = np.asarray(inputs["Wv"], dtype=np.float32)
    Wo = np.asarray(inputs["Wo"], dtype=np.float32)

    hst = np.ascontiguousarray(hidden.T).astype(BF)

    # neox rotate-half RoPE tables, partition p carries frequency p & 63;
    # top half gets -sin so that raw*cos + rot*sinpm == rotate_half rope.
    p = np.arange(128)
    invf = (1e6) ** (-(p & 63) / 64.0)
    ang = invf[:, None] * positions[None, :]
    cosd = np.cos(ang).astype(np.float32)
    sin = np.sin(ang)
    sind = np.concatenate([-sin[:64], sin[64:]], axis=0).astype(np.float32)

    trace = os.environ.get("KERNEL_TRACE", "0") == "1"
    if trace:
        _install_ntff_hook()

    nc = _get_nc()
    in_maps = []
    for c in range(NCORES):
        in_maps.append({
            "hst": hst,
            "wq": np.ascontiguousarray(Wq[:, c * DQ:(c + 1) * DQ]).astype(BF),
            "wk": np.ascontiguousarray(Wk[:, c * D:(c + 1) * D]).astype(BF),
            "wv": np.ascontiguousarray(Wv[:, c * D:(c + 1) * D]).astype(BF),
            "wo": np.ascontiguousarray(Wo[c * DQ:(c + 1) * DQ, :]).astype(BF),
            "cosd": cosd,
            "sind": sind,
        })
    res = run_bass_kernel_spmd(nc, in_maps, core_ids=list(range(NCORES)),
                               trace=trace)
    LAST_EXEC_NS = res.exec_time_ns
    acc = np.zeros((HID, T), dtype=np.float32)
    for c in range(NCORES):
        acc += res.results[c]["yt"].astype(np.float32)
    return np.ascontiguousarray(acc.T)
